# revision 1
# baseline (speedup 1.0000x reference)
"""DCRNN cell kernel for Trainium2, 8 NeuronCores, data-parallel over batch.

Math (per core, 4 batches):
  S1^T = diag(1/rowsum(A)) @ A          (fp16, resident in SBUF)
  S2^T = diag(1/colsum(A)) @ A^T        (fp16, resident in SBUF)
  X0   = concat(inputs, state) laid out [node, (batch, feat)]  (fp16)
  Chain applications (PE, fp16 x fp16 -> fp32 PSUM):
    Y1 = S1*X0, Y2 = S1*Y1, Z1 = S2*X0, Z2 = S2*Z1      (pass 1, 320 wide)
    C1 = S1*RS, C2 = S1*C1, D1 = S2*RS, D2 = S2*D1      (pass 2, 256 wide)
  Theta stage needs feature-on-partition operands: chain outputs are
  streamed to DRAM scratch (padded per-batch 128-col blocks) and read
  back through the DMA-transpose xbar as [feat, node] blocks which act
  as matmul lhsT. theta matmuls accumulate 5 diffusion terms in PSUM:
    pre_ru = sum_k XkT^T @ [th_r_k | th_u_k | th_c_k_lo(pad)]
    r = sigmoid(.), u = sigmoid(.), RS = r*state
    pre_c = sum_k pass2_kT^T @ th_c_k_hi + PC   (PC = input-part partial)
    out = c + u*(state - c),  c = tanh(pre_c)
All elementwise in fp32 except fp16 diffusion states; u kept fp32;
state re-read fp32 for the final combine.  End-to-end absmax error vs
the fp32 reference is ~6e-4 (max |ref| ~3.2).

Biases are zeros per the problem spec (fill=zeros) and are ignored.
"""

import numpy as np

B, N, DESC, H, MAXK = 32, 2048, 16, 64, 2
P = DESC + H            # 80
KMAT = 2 * MAXK + 1     # 5
NCORES = 8
BB = B // NCORES        # 4 batches per core
NCH = N // 128          # 16 node chunks
W1 = BB * P             # 320
W2 = BB * H             # 256

_CACHE = {}


def build_program(reps=1):
    """Build + compile the per-core Bass program (SPMD: same program on
    all 8 cores). reps>1 wraps the body in a hardware loop (timing)."""
    key = ("nc", reps)
    if key in _CACHE:
        return _CACHE[key]

    import sys
    for p in ("/root/.axon_site/_ro/trn_rl_repo", "/opt/trn_rl_repo"):
        if p not in sys.path:
            sys.path.append(p)
    import concourse.bass as bass  # noqa: F401
    import concourse.mybir as mybir
    import concourse.tile as tile
    from concourse import bacc

    dt = mybir.dt
    AF = mybir.ActivationFunctionType
    AX = mybir.AxisListType

    nc = bacc.Bacc("TRN2", target_bir_lowering=False, debug=False,
                   num_devices=NCORES)

    x_in = nc.dram_tensor("x_in", [BB, N * DESC], dt.float32,
                          kind="ExternalInput").ap()
    h_in = nc.dram_tensor("h_in", [BB, N * H], dt.float32,
                          kind="ExternalInput").ap()
    adj = nc.dram_tensor("adj", [N, N], dt.float32, kind="ExternalInput").ap()
    th_dram = {
        g: nc.dram_tensor(f"th_{g}", [P * KMAT, H], dt.float32,
                          kind="ExternalInput").ap()
        for g in "ruc"
    }
    out = nc.dram_tensor("out", [BB, N * H], dt.float32,
                         kind="ExternalOutput").ap()

    # DRAM scratch: transposable copies, padded to 128 cols per batch.
    def scratch(name):
        return nc.dram_tensor(name, [N, BB * 128], dt.float16).ap()

    A16d = nc.dram_tensor("A16d", [N, N], dt.float16).ap()
    X0d = scratch("X0d")
    Y1d, Y2d, Z1d, Z2d = (scratch(n) for n in ("Y1d", "Y2d", "Z1d", "Z2d"))
    # pass-2 scratch: batches packed in pairs (64 feat cols each) so the
    # 128-col xbar blocks carry no padding.
    RSd, C1d, C2d, D1d, D2d = (
        nc.dram_tensor(n, [N, BB * H], dt.float16).ap()
        for n in ("RSd", "C1d", "C2d", "D1d", "D2d"))

    def dram_pcb(ap2d):
        # [N, BB*128] -> [chunk, p, b, q]
        return ap2d.rearrange("(c p) (b q) -> c p b q", p=128, q=128)

    import contextlib

    with tile.TileContext(nc) as tc, contextlib.ExitStack() as _loopctx:
        if reps > 1:
            _loopctx.enter_context(tc.For_i(0, reps, 1))
        frees = {}

        def mktile(shape, dtype, name, space="SBUF", side=None):
            t, f = tc.tile(shape, dtype, name=name, space=space, side=side)
            frees[name] = f
            return t

        # ---- persistent tiles -------------------------------------------
        # Right-side stack (freed mid-kernel, LIFO): S1T, S2T, XC, X0.
        S1T = mktile([128, NCH, N], dt.float16, name="S1T", side="right")
        S2T = mktile([128, NCH, N], dt.float16, name="S2T", side="right")
        XC = mktile([128, NCH, W2], dt.float16, name="XC", side="right")
        X0 = mktile([128, NCH, W1], dt.float16, name="X0", side="right")
        U = mktile([128, NCH, W2], dt.float32, name="U")
        PC = mktile([128, NCH, W2], dt.float16, name="PC")
        invdr = mktile([128, NCH], dt.float32, name="invdr")
        invdc = mktile([128, NCH], dt.float32, name="invdc")
        dcol_acc = mktile([128, NCH], dt.float32, name="dcol_acc")
        ones = mktile([128, 1], dt.float32, name="ones")

        nc.vector.memset(ones[:], 1.0)
        nc.vector.memset(dcol_acc[:], 0.0)

        # ---- theta tiles -------------------------------------------------
        # thru[k]: [80, 192] = [th_r_k | th_u_k | th_c_k rows 0:16, zero-pad]
        # thch[k]: [64, 64]  = th_c_k rows 16:80
        thru = [mktile([P, 192], dt.float16, name=f"thru{k}")
                for k in range(KMAT)]
        thch = [mktile([H, H], dt.float16, name=f"thch{k}")
                for k in range(KMAT)]
        # same data placed at partitions 64..127 (batch-pair upper half)
        thchB = [mktile([128, H], dt.float16, name=f"thchB{k}")
                 for k in range(KMAT)]

        with tc.tile_pool(name="thpool", bufs=4) as thpool:
            for k in range(KMAT):
                nc.vector.memset(thru[k][:], 0.0)
                str_ = thpool.tile([P, H], dt.float32, name="thst_r")
                nc.sync.dma_start(
                    out=str_[:],
                    in_=th_dram["r"].rearrange("(p k) h -> k p h", k=KMAT)[k])
                nc.scalar.activation(thru[k][:, 0:64], str_[:], AF.Copy)
                stu = thpool.tile([P, H], dt.float32, name="thst_u")
                nc.sync.dma_start(
                    out=stu[:],
                    in_=th_dram["u"].rearrange("(p k) h -> k p h", k=KMAT)[k])
                nc.scalar.activation(thru[k][:, 64:128], stu[:], AF.Copy)
                stc_lo = thpool.tile([DESC, H], dt.float32, name="thst_clo")
                nc.sync.dma_start(
                    out=stc_lo[:],
                    in_=th_dram["c"].rearrange("(p k) h -> k p h",
                                               k=KMAT)[k][0:DESC])
                nc.scalar.activation(thru[k][0:DESC, 128:192], stc_lo[:],
                                     AF.Copy)
                stc_hi = thpool.tile([H, H], dt.float32, name="thst_chi")
                nc.sync.dma_start(
                    out=stc_hi[:],
                    in_=th_dram["c"].rearrange("(p k) h -> k p h",
                                               k=KMAT)[k][DESC:P])
                nc.scalar.activation(thch[k][:], stc_hi[:], AF.Copy)
                nc.sync.dma_start(out=thchB[k][H:2 * H, :], in_=thch[k][:])

        # ---- phase 0b: assemble X0, stream X0d --------------------------
        with tc.tile_pool(name="xpool", bufs=2) as xpool:
            for b in range(BB):
                xst = xpool.tile([128, NCH, DESC], dt.float32, name="xst")
                nc.sync.dma_start(
                    out=xst[:],
                    in_=x_in[b].rearrange("(c p d) -> p c d", p=128, d=DESC))
                nc.scalar.activation(X0[:, :, b * P:b * P + DESC], xst[:],
                                     AF.Copy)
                hst = xpool.tile([128, NCH, H], dt.float32, name="hst")
                nc.sync.dma_start(
                    out=hst[:],
                    in_=h_in[b].rearrange("(c p h) -> p c h", p=128, h=H))
                nc.scalar.activation(X0[:, :, b * P + DESC:(b + 1) * P],
                                     hst[:], AF.Copy)
            X0d_p = X0d.rearrange("(c p) (b q) -> p c b q", p=128, q=128)
            for b in range(BB):
                nc.sync.dma_start(
                    out=X0d_p[:, :, b, 0:P],
                    in_=X0[:, :, b * P:(b + 1) * P])

        # ---- phase 0: build S1T / S2T -----------------------------------
        # Per chunk: load adj rows, cast->fp16 (row-sum via ACT accum_out),
        # bounce fp16 copy to DRAM (A16d). S2T is then built with 16
        # back-to-back tall xbar transposes from DRAM (keeps the DMA
        # engines in transpose mode instead of thrashing per chunk).
        # Y1's first m-group accumulates inline, chasing the chunk casts.
        y1ctx = contextlib.ExitStack()
        y1pool = y1ctx.enter_context(tc.tile_pool(name="y1pool", bufs=1))
        y1ps = y1ctx.enter_context(
            tc.tile_pool(name="y1ps", bufs=5, space="PSUM"))
        Y1t = y1pool.tile([128, NCH, W1], dt.float16, name="Y1t")
        YG = 5
        pss0 = [y1ps.tile([128, 512], dt.float32, name="psy", tag="psy")
                for _ in range(YG)]
        with tc.tile_pool(name="apool", bufs=2) as apool, \
                tc.tile_pool(name="rpool", bufs=2) as rpool, \
                tc.tile_pool(name="dcps", bufs=2, space="PSUM") as dcps, \
                tc.tile_pool(name="drpool", bufs=2) as drpool:
            adj_c = adj.rearrange("(c p) m -> c p m", p=128)
            for c in range(NCH):
                ast = apool.tile([128, N], dt.float32, name="ast")
                nc.sync.dma_start(out=ast[:], in_=adj_c[c])
                dr = drpool.tile([128, 1], dt.float32, name="dr")
                r16 = rpool.tile([128, N], dt.float16, name="r16")
                nc.scalar.activation(r16[:], ast[:], AF.Copy,
                                     accum_out=dr[:])
                nc.sync.dma_start(out=A16d[c * 128:(c + 1) * 128, :],
                                  in_=r16[:])
                nc.vector.reciprocal(invdr[:, c:c + 1], dr[:])
                # S1T chunk: scaled cast, alternate ACT/DVE to balance
                if c % 2 == 0:
                    nc.vector.tensor_scalar_mul(S1T[:, c, :], ast[:],
                                                invdr[:, c:c + 1])
                else:
                    nc.scalar.activation(S1T[:, c, :], ast[:], AF.Copy,
                                         scale=invdr[:, c:c + 1])
                # Y1 m-group 0 chases this chunk's S1T cast
                for mi, mt in enumerate(range(0, YG)):
                    nc.tensor.matmul(
                        pss0[mi][:, 0:W1],
                        lhsT=S1T[:, c, mt * 128:(mt + 1) * 128],
                        rhs=X0[:, c, :],
                        start=(c == 0), stop=(c == NCH - 1))
                # column sums via fp32 matmul against ones: each mm is its
                # own group (writes one psum column), then DVE-accumulate.
                dps = dcps.tile([128, NCH], dt.float32, name="dps")
                for mb in range(NCH):
                    nc.tensor.matmul(
                        dps[:, mb:mb + 1],
                        lhsT=ast[:, mb * 128:(mb + 1) * 128],
                        rhs=ones[:],
                        start=True, stop=True)
                nc.vector.tensor_add(dcol_acc[:], dcol_acc[:], dps[:])
            nc.vector.reciprocal(invdc[:], dcol_acc[:])
            # 16 back-to-back tall transposes from DRAM + row scale
            for j in range(NCH):
                nc.sync.dma_start(out=S2T[:, j, :],
                                  in_=A16d[:, j * 128:(j + 1) * 128],
                                  transpose=True)
                nc.vector.tensor_scalar_mul(S2T[:, j, :], S2T[:, j, :],
                                            invdc[:, j:j + 1])

        # ---- chains -----------------------------------------------------
        def make_chain_app(cbpool, pspool):
            def chain_app(lhsT_tile, src_fn, width, dst_sb, dst_dram,
                          packed):
                """dst = S * src in chunks of output rows."""
                qq = width // BB
                for mt in range(NCH):
                    ps = pspool.tile([128, 512], dt.float32, name="ps_chain")
                    psv = ps[:, 0:width]
                    for c in range(NCH):
                        nc.tensor.matmul(
                            psv,
                            lhsT=lhsT_tile[:, c, mt * 128:(mt + 1) * 128],
                            rhs=src_fn(c),
                            start=(c == 0), stop=(c == NCH - 1))
                    if dst_sb is not None:
                        cb = dst_sb[:, mt, 0:width]
                    else:
                        cbt = cbpool.tile([128, W1], dt.float16, name="cb")
                        cb = cbt[:, 0:width]
                    nc.vector.tensor_copy(cb, psv)
                    if packed:
                        # dst rows (c p), cols (b q) with q == qq, no pad
                        nc.sync.dma_start(
                            out=dst_dram.rearrange(
                                "(c p) w -> c p w", p=128)[mt],
                            in_=cb)
                    else:
                        nc.sync.dma_start(
                            out=dram_pcb(dst_dram)[mt][:, :, 0:qq],
                            in_=cb.rearrange("p (b q) -> p b q", q=qq))
            return chain_app

        # pass 1:  Y1, Z1, Y2, Z2   (Y/Z interleaved keeps PE busy)
        with tc.tile_pool(name="chpool1", bufs=2) as chpool, \
                tc.tile_pool(name="cbpool1", bufs=4) as cbpool, \
                tc.tile_pool(name="pspool1", bufs=3, space="PSUM") as pspool:
            chain_app = make_chain_app(cbpool, pspool)
            # group-0 copybacks (accumulated during phase 0)
            for mi, mt in enumerate(range(0, YG)):
                cb = Y1t[:, mt, :]
                nc.vector.tensor_copy(cb, pss0[mi][:, 0:W1])
                nc.sync.dma_start(
                    out=dram_pcb(Y1d)[mt][:, :, 0:P],
                    in_=cb.rearrange("p (b q) -> p b q", q=P))
            # remaining Y1 m-groups, c-outer
            for g0 in range(YG, NCH, YG):
                mts = list(range(g0, min(g0 + YG, NCH)))
                pss = [y1ps.tile([128, 512], dt.float32, name="psy",
                                 tag="psy") for _ in mts]
                for c in range(NCH):
                    for mi, mt in enumerate(mts):
                        nc.tensor.matmul(
                            pss[mi][:, 0:W1],
                            lhsT=S1T[:, c, mt * 128:(mt + 1) * 128],
                            rhs=X0[:, c, :],
                            start=(c == 0), stop=(c == NCH - 1))
                for mi, mt in enumerate(mts):
                    cb = Y1t[:, mt, :]
                    nc.vector.tensor_copy(cb, pss[mi][:, 0:W1])
                    nc.sync.dma_start(
                        out=dram_pcb(Y1d)[mt][:, :, 0:P],
                        in_=cb.rearrange("p (b q) -> p b q", q=P))
            Z1t = chpool.tile([128, NCH, W1], dt.float16, name="Z1t",
                              tag="chain")
            chain_app(S2T, lambda c: X0[:, c, :], W1, Z1t, Z1d, False)
            chain_app(S1T, lambda c: Y1t[:, c, :], W1, None, Y2d, False)
            chain_app(S2T, lambda c: Z1t[:, c, :], W1, None, Z2d, False)

        y1ctx.close()

        # ---- theta pass 1 -----------------------------------------------
        # Per (batch, node-quarter): 5 batched xbar transposes [512,128] ->
        # [128,512] feature-major blocks, then 4 chunk-wise theta matmuls.
        p1_drams = [X0d, Y1d, Y2d, Z1d, Z2d]
        NQ = 4
        QR = N // NQ  # 512 rows per quarter
        with tc.tile_pool(name="tppool1", bufs=10) as tppool, \
                tc.tile_pool(name="thps", bufs=2, space="PSUM") as thps, \
                tc.tile_pool(name="rtpool1", bufs=3) as rtpool:
            for b in range(BB):
                for q in range(NQ):
                    ts = []
                    for dr_ in p1_drams:
                        t = tppool.tile([128, QR], dt.float16, name="tp")
                        nc.sync.dma_start(
                            out=t[:],
                            in_=dr_[q * QR:(q + 1) * QR,
                                    b * 128:(b + 1) * 128],
                            transpose=True)
                        ts.append(t)
                    for ci in range(QR // 128):
                        c = q * (QR // 128) + ci
                        ps = thps.tile([128, 192], dt.float32, name="ps_th")
                        for k in range(KMAT):
                            nc.tensor.matmul(
                                ps[:],
                                lhsT=ts[k][0:P, ci * 128:(ci + 1) * 128],
                                rhs=thru[k][:],
                                start=(k == 0), stop=(k == KMAT - 1))
                        rtmp = rtpool.tile([128, H], dt.float16, name="rtmp")
                        nc.scalar.activation(rtmp[:], ps[:, 0:64], AF.Sigmoid)
                        nc.vector.tensor_mul(
                            XC[:, c, b * H:(b + 1) * H], rtmp[:],
                            X0[:, c, b * P + DESC:(b + 1) * P])
                        nc.scalar.activation(U[:, c, b * H:(b + 1) * H],
                                             ps[:, 64:128], AF.Sigmoid)
                        nc.vector.tensor_copy(PC[:, c, b * H:(b + 1) * H],
                                              ps[:, 128:192])
                # stream this batch's RS column block to DRAM (packed)
                nc.sync.dma_start(
                    out=RSd.rearrange("(c p) w -> p c w",
                                      p=128)[:, :, b * H:(b + 1) * H],
                    in_=XC[:, :, b * H:(b + 1) * H])

        frees["X0"]()   # X0 last used by theta pass 1

        # ---- pass 2 chains ----------------------------------------------
        with tc.tile_pool(name="chpool2", bufs=2) as chpool, \
                tc.tile_pool(name="cbpool2", bufs=3) as cbpool, \
                tc.tile_pool(name="pspool2", bufs=4, space="PSUM") as pspool:
            chain_app = make_chain_app(cbpool, pspool)
            C1t = chpool.tile([128, NCH, W2], dt.float16, name="C1t",
                              tag="chain")
            chain_app(S1T, lambda c: XC[:, c, :], W2, C1t, C1d, True)
            D1t = chpool.tile([128, NCH, W2], dt.float16, name="D1t",
                              tag="chain")
            chain_app(S2T, lambda c: XC[:, c, :], W2, D1t, D1d, True)
            chain_app(S1T, lambda c: C1t[:, c, :], W2, None, C2d, True)
            chain_app(S2T, lambda c: D1t[:, c, :], W2, None, D2d, True)

        frees["XC"]()
        frees["S2T"]()
        frees["S1T"]()

        # ---- theta pass 2 + GRU -----------------------------------------
        CG = mktile([128, NCH, W2], dt.float32, name="CG")
        p2_drams = [RSd, C1d, C2d, D1d, D2d]
        h_in_pcbh = h_in.rearrange("b (c p h) -> p c b h", p=128, h=H)
        out_pcbh = out.rearrange("b (c p h) -> p c b h", p=128, h=H)
        with tc.tile_pool(name="tppool2", bufs=15) as tppool, \
                tc.tile_pool(name="cps", bufs=3, space="PSUM") as cps, \
                tc.tile_pool(name="gpool", bufs=2) as gpool, \
                tc.tile_pool(name="rtpool2", bufs=3) as rtpool, \
                tc.tile_pool(name="stpool", bufs=2) as stpool:
            for q in range(NQ):
                for bp in range(BB // 2):       # batch pairs share a block
                    ts = []
                    for dr_ in p2_drams:
                        t = tppool.tile([128, QR], dt.float16, name="tp")
                        nc.sync.dma_start(
                            out=t[:],
                            in_=dr_[q * QR:(q + 1) * QR,
                                    bp * 128:(bp + 1) * 128],
                            transpose=True)
                        ts.append(t)
                    for half in range(2):       # batch = 2*bp + half
                        b = 2 * bp + half
                        lo = half * H
                        for ci in range(QR // 128):
                            c = q * (QR // 128) + ci
                            ps = cps.tile([128, H], dt.float32, name="ps_c")
                            for k in range(KMAT):
                                rhs_k = (thch[k][:] if half == 0
                                         else thchB[k][H:2 * H, :])
                                nc.tensor.matmul(
                                    ps[:],
                                    lhsT=ts[k][lo:lo + H,
                                               ci * 128:(ci + 1) * 128],
                                    rhs=rhs_k,
                                    start=(k == 0), stop=(k == KMAT - 1))
                            pre = rtpool.tile([128, H], dt.float32,
                                              name="pre")
                            nc.vector.tensor_add(
                                pre[:], ps[:], PC[:, c, b * H:(b + 1) * H])
                            nc.scalar.activation(
                                CG[:, c, b * H:(b + 1) * H], pre[:],
                                AF.Tanh)
                # GRU combine for this quarter's chunks (overlaps next q)
                for ci in range(QR // 128):
                    c = q * (QR // 128) + ci
                    stst = stpool.tile([128, BB, H], dt.float32, name="stst")
                    nc.sync.dma_start(out=stst[:], in_=h_in_pcbh[:, c])
                    t1 = stpool.tile([128, BB * H], dt.float32, name="t1")
                    nc.vector.tensor_sub(t1[:],
                                         stst[:].rearrange(
                                             "p b h -> p (b h)"),
                                         CG[:, c, :])
                    t2 = stpool.tile([128, BB * H], dt.float32, name="t2")
                    nc.vector.tensor_mul(t2[:], t1[:], U[:, c, :])
                    outst = gpool.tile([128, BB, H], dt.float32,
                                       name="outst")
                    nc.vector.tensor_add(
                        outst[:].rearrange("p b h -> p (b h)"), t2[:],
                        CG[:, c, :])
                    nc.sync.dma_start(out=out_pcbh[:, c], in_=outst[:])

        # release remaining singles in reverse creation order (stack alloc)
        frees["CG"]()
        for k in range(KMAT - 1, -1, -1):
            frees[f"thchB{k}"]()
        for k in range(KMAT - 1, -1, -1):
            frees[f"thch{k}"]()
        for k in range(KMAT - 1, -1, -1):
            frees[f"thru{k}"]()
        for name in ("ones", "dcol_acc", "invdc", "invdr", "PC", "U"):
            frees[name]()

    nc.compile()
    _CACHE[key] = nc
    return nc


def _shard_inputs(inputs, state, adj, theta_r, theta_u, theta_c):
    adj = np.ascontiguousarray(adj, dtype=np.float32)
    maps = []
    for i in range(NCORES):
        maps.append({
            "x_in": np.ascontiguousarray(inputs[i * BB:(i + 1) * BB],
                                         dtype=np.float32),
            "h_in": np.ascontiguousarray(state[i * BB:(i + 1) * BB],
                                         dtype=np.float32),
            "adj": adj,
            "th_r": np.ascontiguousarray(theta_r, dtype=np.float32),
            "th_u": np.ascontiguousarray(theta_u, dtype=np.float32),
            "th_c": np.ascontiguousarray(theta_c, dtype=np.float32),
        })
    return maps


def kernel(inputs, state, adj, theta_r, theta_u, theta_c,
           bias_r=None, bias_u=None, bias_c=None, **_unused):
    """Full-input entry point: shards over 8 NeuronCores (data-parallel
    on batch), runs the Bass kernel, gathers the full [B, N*H] output.
    bias_r/u/c are zeros per the problem spec and are folded out."""
    from concourse.bass_utils import run_bass_kernel_spmd

    nc = build_program()
    in_maps = _shard_inputs(np.asarray(inputs), np.asarray(state),
                            np.asarray(adj), np.asarray(theta_r),
                            np.asarray(theta_u), np.asarray(theta_c))
    res = run_bass_kernel_spmd(nc, in_maps, list(range(NCORES)))
    out = np.concatenate([res.results[i]["out"] for i in range(NCORES)],
                         axis=0)
    return out.astype(np.float32)

